# revision 7
# baseline (speedup 1.0000x reference)
"""ConvTranspose3d(64->32, k=3, stride=2, pad=1, out_pad=1, dilation=2) on 8 NeuronCores.

Math: with stride=2, dilation=2, padding=1, k=3, output position o = 2i + 2k - 1
is odd in every spatial dim, so the transposed conv collapses to a dense 3^3
conv y = conv3d(x, wc, padding=1) on the 32^3 grid (wc = flip(transpose(w))),
scattered into the odd sub-lattice of the 66^3 output; every other output
voxel is just bias (host fills those).

Sharding: 8 shards = 2 batches x 4 depth-blocks of 8 conv planes. Implicit
GEMM per core: M = (c_out=32 x 4 depth planes) on PSUM partitions, K =
(64 c_in x 2 input planes) via a block-Toeplitz stationary operand, N = 512
hw pixels, 27 taps x 3 K-chunks per PSUM bank.

fp8 DoubleRow: inputs are decomposed x = h + r, w = wh + wr with each part in
e4m3 at power-of-2 scales (X32=32h, XR=32r, W64=64wh, WR32=64wr), so
y*2048 = X32*W64 + X32*WR32 + XR*W64 exactly (dropping only r*wr ~ 2^-8).
All three terms accumulate in one fp32 PSUM group as 81 K-tiles of 128, run
as 41 DoubleRow passes (2 K-tiles each, 0.5 cycles/row = 2x bf16). Pair
k-tiles must sit at an EVEN SBUF element offset delta (odd deltas hang the
PE), so taps pair as (kw=0,kw=2) and (kw=1,kw=1'). P3 reuses P1's stationary
blocks; P1/P2 share the X32 rhs, so DMA is 1.48MB x + 0.88MB w per core.
"""

import sys

sys.path.insert(0, "/opt/trn_rl_repo")

import numpy as np
import ml_dtypes

N_CORES = 8
D_BLOCKS = 4
G_PER_CORE = 8
E4 = ml_dtypes.float8_e4m3

_cache = {}

# per-term k-tile pairing honoring the even-delta rule; entries are
# (c, kh, kw) and every pass is a 2-list
_PAIRS_EVEN = [((c, kh, 0), (c, kh, 2)) for c in range(3) for kh in range(3)]
_KW1 = [(c, kh, 1) for c in range(3) for kh in range(3) if (c, kh) != (2, 2)]
_PAIRS_KW1 = [(_KW1[2 * i], _KW1[2 * i + 1]) for i in range(4)]
_TERM_PAIRS = _PAIRS_EVEN + _PAIRS_KW1  # 13 pairs; leftover = (2, 2, 1)
_LEFT = (2, 2, 1)


def _phys(term, c, kh, kw):
    # stationary block column: P1/P3 share W64 blocks, P2 uses WR32 blocks,
    # term 3 is the all-zero pad block
    if term == 3:
        return 54
    base = 27 if term == 1 else 0
    return base + c * 9 + kh * 3 + kw


def _pass_list():
    """41 passes; each is ((term0, c0, kh0, kw0), (term1, c1, kh1, kw1))."""
    out = []
    for term in (0, 1):
        out += [((term,) + a, (term,) + b) for a, b in _TERM_PAIRS]
    out.append(((0,) + _LEFT, (1,) + _LEFT))     # P1/P2 leftovers, delta 0
    out += [((2,) + a, (2,) + b) for a, b in _TERM_PAIRS]
    out.append(((2,) + _LEFT, (3, 0, 0, 0)))     # P3 leftover + zero pad
    return out


def _build_nc():
    import concourse.bass as bass
    import concourse.tile as tile
    from concourse import bacc, mybir

    dt = mybir.dt
    DR = mybir.MatmulPerfMode.DoubleRow
    nc = bacc.Bacc("TRN2", target_bir_lowering=False, debug=False,
                   num_devices=N_CORES)

    # xs8[s, p, part, h, w]: s=0 -> X32, s=1 -> XR; pair p planes; part=dpi*64+ci
    xs8 = nc.dram_tensor("xs8", [2, 5, 128, 34, 34], dt.float8e4,
                         kind="ExternalInput")
    # tw8 columns: phys*128: 0..26 -> W64 blocks (c*9+kh*3+kw), 27..53 -> WR32,
    # 54 -> zeros
    tw8 = nc.dram_tensor("tw8", [128, 55 * 128], dt.float8e4,
                         kind="ExternalInput")
    bias = nc.dram_tensor("bias", [128, 1], dt.float32, kind="ExternalInput")
    out = nc.dram_tensor("out", [128, 4, 16, 32], dt.bfloat16,
                         kind="ExternalOutput")

    passes = _pass_list()

    with tile.TileContext(nc) as tc:
        with (
            tc.tile_pool(name="tw", bufs=1) as tw_pool,
            tc.tile_pool(name="xa", bufs=1) as xa_pool,
            tc.tile_pool(name="bias", bufs=1) as bias_pool,
            tc.tile_pool(name="og", bufs=1) as og_pool,
            tc.tile_pool(name="wu", bufs=1) as wu_pool,
            tc.tile_pool(name="psw", bufs=1, space="PSUM") as psw_pool,
            tc.tile_pool(name="ps", bufs=4, space="PSUM") as ps_pool,
        ):
            twt = tw_pool.tile([128, 55 * 128], dt.float8e4)
            xall = xa_pool.tile([128, 2, 5, 34, 34], dt.float8e4)
            bias_t = bias_pool.tile([128, 1], dt.float32)

            # PE p-state warmup on scratch data during the DMA wait
            wul = wu_pool.tile([128, 2, 128], dt.float8e4)
            wur = wu_pool.tile([128, 2, 16, 32], dt.float8e4, tag="wur")
            nc.gpsimd.memset(wul[:], 0)
            nc.gpsimd.memset(wur[:], 0)
            psw = psw_pool.tile([128, 16, 32], dt.float32)
            for _ in range(12):
                nc.tensor.matmul(psw[:], wul[:], wur[:], start=True,
                                 stop=True, perf_mode=DR)

            def ldx(s, plo, phi, r0, r1, eng):
                eng.dma_start(xall[:, s, plo:phi, r0:r1, :],
                              xs8[s, plo:phi, :, r0:r1, :]
                              .rearrange("p q r c -> q p r c"))

            def ldtw(lo, hi, eng):
                eng.dma_start(twt[:, lo * 128:hi * 128],
                              tw8[:, lo * 128:hi * 128])

            # first-use-ordered loads, 3 parallel issue queues
            ldx(0, 0, 1, 0, 18, nc.scalar)    # X32 p0 upper: pass 0 of g0
            ldtw(0, 6, nc.sync)               # first stationary blocks
            ldtw(27, 55, nc.gpsimd)           # P2 blocks (needed pass 13)
            ldtw(6, 27, nc.sync)              # P1 mid blocks
            ldx(0, 1, 3, 0, 18, nc.scalar)    # X32 p1-2 upper
            ldx(0, 0, 3, 18, 34, nc.scalar)   # X32 p0-2 lower (g1)
            ldx(1, 0, 3, 0, 34, nc.gpsimd)    # XR p0-2 (P3 of g0/g1)
            nc.sync.dma_start(bias_t[:], bias[:])
            ldx(0, 3, 5, 0, 34, nc.gpsimd)    # X32 p3-4 (g2/g3)
            ldx(1, 3, 5, 0, 34, nc.gpsimd)    # XR p3-4

            XSTR = 2 * 5 * 1156  # xall per-partition elements
            WSTR = 55 * 128

            def koff(b, hh, term, c, kh, kw):
                s = 1 if term == 2 else 0
                return (s * 5 * 1156 + (2 * b + c) * 1156
                        + (16 * hh + kh) * 34 + kw)

            prev_last_mm = None
            for b in range(2):
                for hh in range(2):
                    g = 2 * b + hh
                    ps = ps_pool.tile([128, 16, 32], dt.float32)
                    for j, (k0, k1) in enumerate(passes):
                        o0 = koff(b, hh, *k0)
                        # zero-pad k-tile repeats k0's rhs (zero weights)
                        o1 = o0 if k1[0] == 3 else koff(b, hh, *k1)
                        dx = o1 - o0
                        assert dx % 2 == 0 and dx >= 0, (k0, k1, dx)
                        base = xall[:, 0, 0, 0:16, 0:32]
                        rhs = bass.AP(base.tensor, base.offset - 0 + o0,
                                      [[XSTR, 128], [dx, 2], [34, 16],
                                       [1, 32]])
                        p0 = _phys(*k0)
                        p1 = _phys(*k1)
                        wbase = twt[:, p0 * 128:p0 * 128 + 128]
                        lhsT = bass.AP(wbase.tensor, wbase.offset,
                                       [[WSTR, 128], [(p1 - p0) * 128, 2],
                                        [1, 128]])
                        mm = nc.tensor.matmul(ps[:], lhsT, rhs,
                                              start=(j == 0), stop=(j == 40),
                                              perf_mode=DR)
                        if j == 0 and prev_last_mm is not None:
                            tile.add_dep_helper(
                                mm.ins, prev_last_mm.ins, sync=False,
                                reason="group-contiguous PE order")
                    prev_last_mm = mm
                    og = og_pool.tile([128, 16, 32], dt.bfloat16,
                                      tag=f"og{g}")
                    nc.vector.tensor_scalar(og[:], ps[:], 1.0 / 2048,
                                            bias_t[:], mybir.AluOpType.mult,
                                            mybir.AluOpType.add)
                    if g < 3:
                        eng = nc.sync if g % 2 == 0 else nc.scalar
                        eng.dma_start(out[:, g], og[:])
                    else:
                        nc.sync.dma_start(out[:, g, 0:8], og[:, 0:8])
                        nc.scalar.dma_start(out[:, g, 8:16], og[:, 8:16])

    nc.compile()
    return nc


def _q8(a):
    return np.clip(a, -240.0, 240.0).astype(E4)


def _prep_shared(weight, bias):
    # wc[co, ci, kd, kh, kw] = weight[ci, co, 2-kd, 2-kh, 2-kw]
    wc = np.flip(np.transpose(weight, (1, 0, 2, 3, 4)), axis=(2, 3, 4))
    # block-Toeplitz: tcw[dpi*64+ci, c*9+t, co*4+gb]
    tcw = np.zeros((128, 27, 128), np.float32)
    for c in range(3):
        for dpi in range(2):
            for gb in range(4):
                kd = 2 * c + dpi - gb
                if 0 <= kd <= 2:
                    arr = wc[:, :, kd].reshape(32, 64, 9).transpose(1, 2, 0)
                    tcw[dpi * 64:(dpi + 1) * 64,
                        c * 9:(c + 1) * 9, gb::4] = arr
    w64 = _q8(64.0 * tcw)
    wh = w64.astype(np.float32) / 64.0
    wr32 = _q8(64.0 * (tcw - wh))
    tw8 = np.zeros((128, 55, 128), E4)
    tw8[:, 0:27] = w64
    tw8[:, 27:54] = wr32
    tw8 = np.ascontiguousarray(tw8.reshape(128, 55 * 128))
    bias128 = np.ascontiguousarray(
        np.repeat(bias.astype(np.float32), 4).reshape(128, 1))
    return tw8, bias128


def _make_slab(x, n, cblk):
    # xs8[s, p, part, h, w]: pairs of padded planes; s=0: X32=q8(32x),
    # s=1: XR=q8(32*(x - X32/32))
    xs = np.zeros((2, 5, 128, 34, 34), E4)
    lo = G_PER_CORE * cblk - 1
    for p in range(5):
        for dpi in range(2):
            d = lo + 2 * p + dpi
            if 0 <= d < 32:
                sl = x[n, :, d].astype(np.float32)
                x32 = _q8(32.0 * sl)
                xr = _q8(32.0 * (sl - x32.astype(np.float32) / 32.0))
                xs[0, p, dpi * 64:(dpi + 1) * 64, 1:33, 1:33] = x32
                xs[1, p, dpi * 64:(dpi + 1) * 64, 1:33, 1:33] = xr
    return xs


def kernel(x, weight, bias):
    from concourse.bass_utils import run_bass_kernel_spmd

    if "nc" not in _cache:
        _cache["nc"] = _build_nc()
    nc = _cache["nc"]

    x = np.asarray(x, np.float32)
    weight = np.asarray(weight, np.float32)
    bias = np.asarray(bias, np.float32)

    tw8, bias128 = _prep_shared(weight, bias)
    in_maps = []
    for core in range(N_CORES):
        n, cblk = divmod(core, D_BLOCKS)
        in_maps.append({"xs8": _make_slab(x, n, cblk), "tw8": tw8,
                        "bias": bias128})

    res = run_bass_kernel_spmd(nc, in_maps, core_ids=list(range(N_CORES)))

    # even sub-lattice (any even coordinate) is pure bias; conv results live
    # on the odd lattice [1:64:2]^3 of the 66^3 volume
    full = np.empty((2, 32, 66, 66, 66), np.float32)
    full[:] = bias[None, :, None, None, None]
    for core in range(N_CORES):
        n, cblk = divmod(core, D_BLOCKS)
        arr = res.results[core]["out"].astype(np.float32)  # (128,4,16,32)
        conv = (arr.reshape(32, 4, 2, 2, 16, 32)
                .transpose(0, 2, 1, 3, 4, 5)   # [co, b, gb, hh, h16, w]
                .reshape(32, 8, 32, 32))
        full[n, :, 16 * cblk + 1:16 * cblk + 17:2, 1:64:2, 1:64:2] = conv
    return full


# revision 9
# speedup vs baseline: 1.2643x; 1.2643x over previous
"""ConvTranspose3d(64->32, k=3, stride=2, pad=1, out_pad=1, dilation=2) on 8 NeuronCores.

Math: with stride=2, dilation=2, padding=1, k=3, output position o = 2i + 2k - 1
is odd in every spatial dim, so the transposed conv collapses to a dense 3^3
conv y = conv3d(x, wc, padding=1) on the 32^3 grid (wc = flip(transpose(w))),
scattered into the odd sub-lattice of the 66^3 output; every other output
voxel is just bias (host fills those).

Sharding: 8 shards = 2 batches x 4 depth-blocks of 8 conv planes. Implicit
GEMM per core: M = (c_out=32 x 4 depth planes) on PSUM partitions, K =
(64 c_in x 2 input planes) via a block-Toeplitz stationary operand (bf16),
N = 512 hw pixels per matmul, 27 matmuls (9 hw taps x 3 K-chunks) per PSUM
bank. 27 passes/group is row-optimal: every K-row (plane, ci) is needed by
some output, and a pass is N-cycle-bound (~216ns) regardless of dtype/K.

Device writes only the 32^3 conv lattice (bf16, [128, 4, 16, 32] per core);
host broadcasts bias into the even sub-lattice. Warmup matmuls on scratch
SBUF ride out the PE p-state ramp during the initial DMA wait.
"""

import sys

sys.path.insert(0, "/opt/trn_rl_repo")

import numpy as np
import ml_dtypes

N_CORES = 8
D_BLOCKS = 4  # depth blocks per batch
G_PER_CORE = 8  # conv output planes per core

_cache = {}


def _build_nc():
    import concourse.bass as bass
    import concourse.tile as tile
    from concourse import bacc, mybir

    dt = mybir.dt
    nc = bacc.Bacc("TRN2", target_bir_lowering=False, debug=False,
                   num_devices=N_CORES)

    # xs: 5 pairs of adjacent (zero-padded) input depth planes; partition
    # p = dpi*64 + ci. tcw: 27 block-Toeplitz stationary matrices, columns
    # (chunk*9 + tap)*128 + (co*4 + gb). bias128: p = co*4+j -> bias[co].
    xs = nc.dram_tensor("xs", [5, 128, 34, 34], dt.bfloat16,
                        kind="ExternalInput")
    tcw = nc.dram_tensor("tcw", [128, 27 * 128], dt.bfloat16,
                         kind="ExternalInput")
    bias = nc.dram_tensor("bias", [128, 1], dt.float32, kind="ExternalInput")
    # compact conv-lattice output: [co*4+gb, group g=2b+hh, h16, w]
    out = nc.dram_tensor("out", [128, 4, 16, 32], dt.bfloat16,
                         kind="ExternalOutput")

    with tile.TileContext(nc) as tc:
        with (
            tc.tile_pool(name="tw", bufs=1) as tw_pool,
            tc.tile_pool(name="xp", bufs=1) as xp_pool,
            tc.tile_pool(name="bias", bufs=1) as bias_pool,
            tc.tile_pool(name="og", bufs=1) as og_pool,
            tc.tile_pool(name="wu", bufs=1) as wu_pool,
            tc.tile_pool(name="psw", bufs=1, space="PSUM") as psw_pool,
            tc.tile_pool(name="ps", bufs=4, space="PSUM") as ps_pool,
        ):
            tw_t = tw_pool.tile([128, 27 * 128], dt.bfloat16)
            xp = []
            for p in range(5):
                xp_tile = xp_pool.tile([128, 34, 34], dt.bfloat16,
                                       tag=f"xp{p}")
                xp.append(xp_tile)
            bias_t = bias_pool.tile([128, 1], dt.float32)

            # PE p-state warmup on scratch data during the initial DMA wait
            wul = wu_pool.tile([128, 128], dt.bfloat16)
            wur = wu_pool.tile([128, 16, 32], dt.bfloat16, tag="wur")
            nc.gpsimd.memset(wul[:], 0)
            nc.gpsimd.memset(wur[:], 0)
            psw = psw_pool.tile([128, 16, 32], dt.float32)
            for _ in range(6):
                nc.tensor.matmul(psw[:], wul[:], wur[:], start=True,
                                 stop=True)

            # first-use-ordered loads on 3 issue queues (sync/scalar/gpsimd)
            nc.scalar.dma_start(xp[0][:, 0:18, :], xs[0, :, 0:18, :])
            nc.sync.dma_start(tw_t[:, 0:2 * 128], tcw[:, 0:2 * 128])
            nc.scalar.dma_start(xp[1][:, 0:18, :], xs[1, :, 0:18, :])
            nc.sync.dma_start(tw_t[:, 2 * 128:8 * 128],
                              tcw[:, 2 * 128:8 * 128])
            nc.scalar.dma_start(xp[2][:, 0:18, :], xs[2, :, 0:18, :])
            nc.sync.dma_start(tw_t[:, 8 * 128:18 * 128],
                              tcw[:, 8 * 128:18 * 128])
            # lower halves of pairs 0-2 (for hh=1 groups), one issue
            for p in range(3):
                nc.scalar.dma_start(xp[p][:, 18:34, :], xs[p, :, 18:34, :])
            nc.sync.dma_start(tw_t[:, 18 * 128:27 * 128],
                              tcw[:, 18 * 128:27 * 128])
            nc.sync.dma_start(bias_t[:], bias[:])
            # pairs 3-4 full (for b=1 groups) on the gpsimd queue
            nc.gpsimd.dma_start(xp[3][:], xs[3])
            nc.gpsimd.dma_start(xp[4][:], xs[4])

            prev_last_mm = None
            for b in range(2):
                for hh in range(2):
                    g = 2 * b + hh
                    h0 = 16 * hh
                    ps = ps_pool.tile([128, 16, 32], dt.float32)
                    i = 0
                    for c in range(3):
                        src = xp[2 * b + c]
                        for t9 in range(9):
                            kh, kw = t9 // 3, t9 % 3
                            lhsT = tw_t[:, (c * 9 + t9) * 128:
                                        (c * 9 + t9 + 1) * 128]
                            rhs = src[:, h0 + kh:h0 + kh + 16, kw:kw + 32]
                            mm = nc.tensor.matmul(ps[:], lhsT, rhs,
                                                  start=(i == 0),
                                                  stop=(i == 26))
                            # keep the PE's static order group-contiguous so
                            # each flush fires right after its 27th matmul
                            if i == 0 and prev_last_mm is not None:
                                tile.add_dep_helper(
                                    mm.ins, prev_last_mm.ins, sync=False,
                                    reason="group-contiguous PE order")
                            i += 1
                    prev_last_mm = mm
                    og = og_pool.tile([128, 16, 32], dt.bfloat16,
                                      tag=f"og{g}")
                    nc.vector.tensor_scalar_add(og[:], ps[:], bias_t[:])
                    if g < 3:
                        eng = nc.sync if g % 2 == 0 else nc.scalar
                        eng.dma_start(out[:, g], og[:])
                    else:
                        # split the tail flush across two queues
                        nc.sync.dma_start(out[:, g, 0:8], og[:, 0:8])
                        nc.scalar.dma_start(out[:, g, 8:16], og[:, 8:16])

    nc.compile()
    return nc


def _prep_shared(weight, bias):
    # wc[co, ci, kd, kh, kw] = weight[ci, co, 2-kd, 2-kh, 2-kw]
    wc = np.flip(np.transpose(weight, (1, 0, 2, 3, 4)), axis=(2, 3, 4))
    # full pre-built Toeplitz: tcw[dpi*64+ci, (c*9+t)*128 + co*4 + gb]
    tcw = np.zeros((128, 27, 128), np.float32)
    for c in range(3):
        for dpi in range(2):
            for gb in range(4):
                kd = 2 * c + dpi - gb
                if 0 <= kd <= 2:
                    arr = wc[:, :, kd].reshape(32, 64, 9).transpose(1, 2, 0)
                    tcw[dpi * 64:(dpi + 1) * 64,
                        c * 9:(c + 1) * 9, gb::4] = arr
    tcw = np.ascontiguousarray(
        tcw.reshape(128, 27 * 128).astype(ml_dtypes.bfloat16))
    bias128 = np.ascontiguousarray(
        np.repeat(bias.astype(np.float32), 4).reshape(128, 1))
    return tcw, bias128


def _make_slab(x, n, cblk):
    # 5 pairs of spatially padded planes (34x34, zero border);
    # pair p = unpadded planes (8c-1+2p, 8c+2p)
    xs = np.zeros((5, 128, 34, 34), ml_dtypes.bfloat16)
    lo = G_PER_CORE * cblk - 1
    for p in range(5):
        for dpi in range(2):
            d = lo + 2 * p + dpi
            if 0 <= d < 32:
                xs[p, dpi * 64:(dpi + 1) * 64, 1:33, 1:33] = \
                    x[n, :, d].astype(ml_dtypes.bfloat16)
    return xs


def kernel(x, weight, bias):
    from concourse.bass_utils import run_bass_kernel_spmd

    if "nc" not in _cache:
        _cache["nc"] = _build_nc()
    nc = _cache["nc"]

    x = np.asarray(x, np.float32)
    weight = np.asarray(weight, np.float32)
    bias = np.asarray(bias, np.float32)

    tcw, bias128 = _prep_shared(weight, bias)
    in_maps = []
    for core in range(N_CORES):
        n, cblk = divmod(core, D_BLOCKS)
        in_maps.append({"xs": _make_slab(x, n, cblk), "tcw": tcw,
                        "bias": bias128})

    res = run_bass_kernel_spmd(nc, in_maps, core_ids=list(range(N_CORES)))

    # even sub-lattice (any even coordinate) is pure bias; conv results live
    # on the odd lattice [1:64:2]^3 of the 66^3 volume
    full = np.empty((2, 32, 66, 66, 66), np.float32)
    full[:] = bias[None, :, None, None, None]
    for core in range(N_CORES):
        n, cblk = divmod(core, D_BLOCKS)
        arr = res.results[core]["out"].astype(np.float32)  # (128,4,16,32)
        conv = (arr.reshape(32, 4, 2, 2, 16, 32)
                .transpose(0, 2, 1, 3, 4, 5)   # [co, b, gb, hh, h16, w]
                .reshape(32, 8, 32, 32))
        full[n, :, 16 * cblk + 1:16 * cblk + 17:2, 1:64:2, 1:64:2] = conv
    return full


# revision 12
# speedup vs baseline: 1.3416x; 1.0611x over previous
"""ConvTranspose3d(64->32, k=3, stride=2, pad=1, out_pad=1, dilation=2) on 8 NeuronCores.

Math: with stride=2, dilation=2, padding=1, k=3, output position o = 2i + 2k - 1
is odd in every spatial dim, so the transposed conv collapses to a dense 3^3
conv y = conv3d(x, wc, padding=1) on the 32^3 grid (wc = flip(transpose(w))),
scattered into the odd sub-lattice of the 66^3 output; every other output
voxel is just bias (host fills those).

Sharding: 8 shards = 2 batches x 4 depth-blocks of 8 conv planes. Implicit
GEMM per core: M = (c_out=32 x 4 depth planes) on PSUM partitions, K =
(64 c_in x 2 input planes) via a block-Toeplitz stationary operand (bf16),
N = 512 hw pixels per matmul, 27 matmuls (9 hw taps x 3 K-chunks) per PSUM
bank. 27 passes/group is row-optimal: every K-row (plane, ci) is needed by
some output, and a pass is N-cycle-bound (~216ns) regardless of dtype/K.

Device writes only the 32^3 conv lattice (bf16, [128, 4, 16, 32] per core);
host broadcasts bias into the even sub-lattice. Warmup matmuls on scratch
SBUF ride out the PE p-state ramp during the initial DMA wait.
"""

import sys

sys.path.insert(0, "/opt/trn_rl_repo")

import numpy as np
import ml_dtypes

N_CORES = 8
D_BLOCKS = 4  # depth blocks per batch
G_PER_CORE = 8  # conv output planes per core

_cache = {}


def _build_nc():
    import concourse.bass as bass
    import concourse.tile as tile
    from concourse import bacc, mybir

    dt = mybir.dt
    nc = bacc.Bacc("TRN2", target_bir_lowering=False, debug=False,
                   num_devices=N_CORES)

    # xs: 5 pairs of adjacent (zero-padded) input depth planes; partition
    # p = dpi*64 + ci. tcw: 27 block-Toeplitz stationary matrices, columns
    # (chunk*9 + tap)*128 + (co*4 + gb). bias128: p = co*4+j -> bias[co].
    xs = nc.dram_tensor("xs", [5, 128, 34, 34], dt.bfloat16,
                        kind="ExternalInput")
    tcw = nc.dram_tensor("tcw", [128, 27 * 128], dt.bfloat16,
                         kind="ExternalInput")
    bias = nc.dram_tensor("bias", [128, 1], dt.float32, kind="ExternalInput")
    # compact conv-lattice output: [co*4+gb, group g=2b+hh, h16, w]
    out = nc.dram_tensor("out", [128, 4, 16, 32], dt.bfloat16,
                         kind="ExternalOutput")

    with tile.TileContext(nc) as tc:
        with (
            tc.tile_pool(name="tw", bufs=1) as tw_pool,
            tc.tile_pool(name="xp", bufs=1) as xp_pool,
            tc.tile_pool(name="bias", bufs=1) as bias_pool,
            tc.tile_pool(name="og", bufs=1) as og_pool,
            tc.tile_pool(name="wu", bufs=1) as wu_pool,
            tc.tile_pool(name="psw", bufs=1, space="PSUM") as psw_pool,
            tc.tile_pool(name="ps", bufs=4, space="PSUM") as ps_pool,
        ):
            tw_t = tw_pool.tile([128, 27 * 128], dt.bfloat16)
            xpt = xp_pool.tile([128, 5, 34, 34], dt.bfloat16)
            bias_t = bias_pool.tile([128, 1], dt.float32)

            # PE p-state warmup on scratch data during the initial DMA wait
            wul = wu_pool.tile([128, 128], dt.bfloat16)
            wur = wu_pool.tile([128, 16, 32], dt.bfloat16, tag="wur")
            nc.gpsimd.memset(wul[:], 0)
            nc.gpsimd.memset(wur[:], 0)
            psw = psw_pool.tile([128, 16, 32], dt.float32)
            for _ in range(6):
                nc.tensor.matmul(psw[:], wul[:], wur[:], start=True,
                                 stop=True)

            def ldx(plo, phi, r0, r1, eng):
                eng.dma_start(xpt[:, plo:phi, r0:r1, :],
                              xs[plo:phi, :, r0:r1, :]
                              .rearrange("p q r c -> q p r c"))

            def ldtw(lo, hi, eng):
                eng.dma_start(tw_t[:, lo * 128:hi * 128],
                              tcw[:, lo * 128:hi * 128])

            # first-use-ordered loads: tcw split across sync+gpsimd in
            # parallel (one queue can't keep pace with the matmul stream),
            # xp pairs 3-4 deferred behind the critical pieces
            ldx(0, 1, 0, 18, nc.scalar)   # pass 0 rhs
            ldtw(0, 2, nc.sync)           # pass 0-1 weights
            ldtw(2, 8, nc.gpsimd)
            ldx(1, 3, 0, 18, nc.scalar)   # passes 9-26 rhs
            ldtw(8, 18, nc.sync)
            ldtw(18, 27, nc.gpsimd)
            ldx(0, 3, 18, 34, nc.scalar)  # hh=1 groups
            nc.sync.dma_start(bias_t[:], bias[:])
            ldx(3, 5, 0, 34, nc.sync)     # b=1 groups

            prev_last_mm = None
            for b in range(2):
                for hh in range(2):
                    g = 2 * b + hh
                    h0 = 16 * hh
                    ps = ps_pool.tile([128, 16, 32], dt.float32)
                    i = 0
                    for c in range(3):
                        for t9 in range(9):
                            kh, kw = t9 // 3, t9 % 3
                            lhsT = tw_t[:, (c * 9 + t9) * 128:
                                        (c * 9 + t9 + 1) * 128]
                            rhs = xpt[:, 2 * b + c,
                                      h0 + kh:h0 + kh + 16, kw:kw + 32]
                            mm = nc.tensor.matmul(ps[:], lhsT, rhs,
                                                  start=(i == 0),
                                                  stop=(i == 26))
                            # keep the PE's static order group-contiguous so
                            # each flush fires right after its 27th matmul
                            if i == 0 and prev_last_mm is not None:
                                tile.add_dep_helper(
                                    mm.ins, prev_last_mm.ins, sync=False,
                                    reason="group-contiguous PE order")
                            i += 1
                    prev_last_mm = mm
                    og = og_pool.tile([128, 16, 32], dt.bfloat16,
                                      tag=f"og{g}")
                    nc.vector.tensor_scalar_add(og[:], ps[:], bias_t[:])
                    if g < 3:
                        eng = nc.sync if g % 2 == 0 else nc.scalar
                        eng.dma_start(out[:, g], og[:])
                    else:
                        # split the tail flush across two queues
                        nc.sync.dma_start(out[:, g, 0:8], og[:, 0:8])
                        nc.scalar.dma_start(out[:, g, 8:16], og[:, 8:16])

    nc.compile()
    return nc


def _prep_shared(weight, bias):
    # wc[co, ci, kd, kh, kw] = weight[ci, co, 2-kd, 2-kh, 2-kw]
    wc = np.flip(np.transpose(weight, (1, 0, 2, 3, 4)), axis=(2, 3, 4))
    # full pre-built Toeplitz: tcw[dpi*64+ci, (c*9+t)*128 + co*4 + gb]
    tcw = np.zeros((128, 27, 128), np.float32)
    for c in range(3):
        for dpi in range(2):
            for gb in range(4):
                kd = 2 * c + dpi - gb
                if 0 <= kd <= 2:
                    arr = wc[:, :, kd].reshape(32, 64, 9).transpose(1, 2, 0)
                    tcw[dpi * 64:(dpi + 1) * 64,
                        c * 9:(c + 1) * 9, gb::4] = arr
    tcw = np.ascontiguousarray(
        tcw.reshape(128, 27 * 128).astype(ml_dtypes.bfloat16))
    bias128 = np.ascontiguousarray(
        np.repeat(bias.astype(np.float32), 4).reshape(128, 1))
    return tcw, bias128


def _make_slab(x, n, cblk):
    # 5 pairs of spatially padded planes (34x34, zero border);
    # pair p = unpadded planes (8c-1+2p, 8c+2p)
    xs = np.zeros((5, 128, 34, 34), ml_dtypes.bfloat16)
    lo = G_PER_CORE * cblk - 1
    for p in range(5):
        for dpi in range(2):
            d = lo + 2 * p + dpi
            if 0 <= d < 32:
                xs[p, dpi * 64:(dpi + 1) * 64, 1:33, 1:33] = \
                    x[n, :, d].astype(ml_dtypes.bfloat16)
    return xs


def kernel(x, weight, bias):
    from concourse.bass_utils import run_bass_kernel_spmd

    if "nc" not in _cache:
        _cache["nc"] = _build_nc()
    nc = _cache["nc"]

    x = np.asarray(x, np.float32)
    weight = np.asarray(weight, np.float32)
    bias = np.asarray(bias, np.float32)

    tcw, bias128 = _prep_shared(weight, bias)
    in_maps = []
    for core in range(N_CORES):
        n, cblk = divmod(core, D_BLOCKS)
        in_maps.append({"xs": _make_slab(x, n, cblk), "tcw": tcw,
                        "bias": bias128})

    res = run_bass_kernel_spmd(nc, in_maps, core_ids=list(range(N_CORES)))

    # even sub-lattice (any even coordinate) is pure bias; conv results live
    # on the odd lattice [1:64:2]^3 of the 66^3 volume
    full = np.empty((2, 32, 66, 66, 66), np.float32)
    full[:] = bias[None, :, None, None, None]
    for core in range(N_CORES):
        n, cblk = divmod(core, D_BLOCKS)
        arr = res.results[core]["out"].astype(np.float32)  # (128,4,16,32)
        conv = (arr.reshape(32, 4, 2, 2, 16, 32)
                .transpose(0, 2, 1, 3, 4, 5)   # [co, b, gb, hh, h16, w]
                .reshape(32, 8, 32, 32))
        full[n, :, 16 * cblk + 1:16 * cblk + 17:2, 1:64:2, 1:64:2] = conv
    return full
